# revision 41
# baseline (speedup 1.0000x reference)
"""CnnReservoirPolicy Trainium2 kernel (8-core SPMD).

Sharding: batch element b -> core b (32 CNN frames/core). BatchNorm batch
statistics (training mode, over the full 256-frame batch) are the only
cross-core CNN dependency: 4 tiny AllReduces. The readout head hW1 is
sharded by output rows (64 rows/core); each core computes its own batch
element's partial H, masked into a [64, 8] rank-column buffer and
AllReduced so every core gets the full 512-dim hidden vector.

conv1 (7x7 s2) runs as a GEMM with M = 64ch x 2 output-row-parity and
K = 189 = (3c x 9yrel x 7kx); its im2col tensor is built host-side.
conv2 packs kx-pairs into K=128 via an x-shifted duplicate of its input.
All matmuls bf16 with fp32 PSUM accumulation. maxpool runs on raw conv1
outputs (commutes with the positive-scale BN affine + relu).

The reservoir scan runs in s = 2r form (0.5*Wres folded host-side, and
0.5*hW1 for the readout) so the update is one scalar_tensor_tensor:
s_t = 0.5*s_{t-1} + tanh(xin_t + (0.5*Wres) s_{t-1}).
"""
import os
import sys

for _p in ("/opt/trn_rl_repo", os.path.expanduser("~/.axon_site/_ro/trn_rl_repo")):
    if os.path.isdir(_p) and _p not in sys.path:
        sys.path.insert(0, _p)

import numpy as np
import ml_dtypes

import concourse.bass as bass
import concourse.mybir as mybir
import concourse.tile as tile
from concourse import bacc
from concourse.bass_utils import run_bass_kernel_spmd

BF16 = ml_dtypes.bfloat16
F32 = mybir.dt.float32
BF = mybir.dt.bfloat16
AX = mybir.AxisListType
ALU = mybir.AluOpType
AF = mybir.ActivationFunctionType

ALPHA = 0.5
BN_EPS = 1e-5
DEBUG_DUMPS = False
NI, T = 2, 16          # num_images, n_obs_steps
FPC = NI * T           # frames per core = 32
K1 = 189               # conv1 GEMM contraction (3c x 9yrel x 7kx)
K1A = 128
K1B = K1 - K1A         # 61


# ----------------------------------------------------------------- host packing

def _bf(x):
    return np.ascontiguousarray(np.asarray(x, np.float32)).astype(BF16)


def conv1_q(imgs):
    """imgs (F,3,112,112) f32 -> Q (108, F, 28, 58) bf16.

    Q[((q*2+par)*3+c)*9+yr, f, r, u] = pad3(imgs)[f, c, 4r+yr, 2(u+q)+par].
    Pass p reads the x-window [2p : 2p+56] and contributes tap kx=2q+par+4p;
    output row oy = 2r+ry pairs with weight column block ry.
    """
    F = imgs.shape[0]
    pb = np.pad(np.asarray(imgs, np.float32),
                ((0, 0), (0, 0), (3, 3), (3, 3))).astype(BF16)
    Q = np.empty((2, 2, 3, 9, F, 28, 58), dtype=BF16)
    for q in range(2):
        for par in range(2):
            x0 = 2 * q + par
            for yr in range(9):
                Q[q, par, :, yr] = pb[:, :, yr:yr + 112:4,
                                      x0:x0 + 116:2].transpose(1, 0, 2, 3)
    return Q.reshape(108, F, 28, 58)


def pack_w1_d4(w1):
    """w1 (64,3,7,7) -> 2 passes of [108, 128]; col = ry*64+co."""
    w1 = np.asarray(w1, np.float32)
    W = np.zeros((2, 2, 2, 3, 9, 128), dtype=np.float32)
    for p_ in range(2):
        for q in range(2):
            for par in range(2):
                kx = 2 * q + par + 4 * p_
                if kx > 6:
                    continue
                for ry in range(2):
                    for ky in range(7):
                        yr = 2 * ry + ky
                        W[p_, q, par, :, yr, ry * 64:(ry + 1) * 64] = \
                            w1[:, :, ky, kx].T
    Wf = W.reshape(2, 108, 128)
    return _bf(Wf[0]), _bf(Wf[1])


def pack_w2(w2):
    """w2 (128,64,3,3) -> A [128=(d,ci),3ky,128co], B [64ci,3ky,128co]."""
    w2 = np.asarray(w2, np.float32)
    wA = np.zeros((128, 3, 128), dtype=np.float32)
    wB = np.zeros((64, 3, 128), dtype=np.float32)
    for ky in range(3):
        for d in range(2):
            wA[d * 64:(d + 1) * 64, ky, :] = w2[:, :, ky, d].T
        wB[:, ky, :] = w2[:, :, ky, 2].T
    return _bf(wA), _bf(wB)


def pack_w3(w3):
    """w3 (256,128,3,3) -> [128ci, 9tap, 2m, 128co]."""
    w3 = np.asarray(w3, np.float32)
    wp = np.empty((128, 9, 2, 128), dtype=np.float32)
    for tap in range(9):
        ky, kx = tap // 3, tap % 3
        for m in range(2):
            wp[:, tap, m, :] = w3[m * 128:(m + 1) * 128, :, ky, kx].T
    return _bf(wp)


def pack_w4(w4):
    """w4 (512,256,3,3) -> [128ci, 9tap, 2h, 4m, 128co]."""
    w4 = np.asarray(w4, np.float32)
    wp = np.empty((128, 9, 2, 4, 128), dtype=np.float32)
    for tap in range(9):
        ky, kx = tap // 3, tap % 3
        for h in range(2):
            for m in range(4):
                wp[:, tap, h, m, :] = \
                    w4[m * 128:(m + 1) * 128, h * 128:(h + 1) * 128, ky, kx].T
    return _bf(wp)


def pack_win(Win):
    """Win (1024,576) -> img [128, 4ki, 8m, 128], state [64, 8m, 128]."""
    Win = np.asarray(Win, np.float32)
    wi = np.empty((128, 4, 8, 128), dtype=np.float32)
    ws = np.empty((64, 8, 128), dtype=np.float32)
    for m in range(8):
        for ki in range(4):
            wi[:, ki, m, :] = Win[m * 128:(m + 1) * 128, ki * 128:(ki + 1) * 128].T
        ws[:, m, :] = Win[m * 128:(m + 1) * 128, 512:].T
    return _bf(wi), _bf(ws)


def pack_wres(Wres):
    """0.5*Wres (1024,1024) -> [128, 8k, 8m, 128]: [p,k,m,q] = W[m*128+q, k*128+p]."""
    Wres = 0.5 * np.asarray(Wres, np.float32)
    wp = np.empty((128, 8, 8, 128), dtype=np.float32)
    for k in range(8):
        for m in range(8):
            wp[:, k, m, :] = Wres[m * 128:(m + 1) * 128, k * 128:(k + 1) * 128].T
    return _bf(wp)


def pack_hw1_slice(hW1, core):
    """0.5*hW1 rows [64c:64c+64) -> [128p, 16t, 8m, 64j]:
    [p,t,m,j] = 0.5*hW1[64c+j, t*1024 + m*128 + p]."""
    hW1 = 0.5 * np.asarray(hW1, np.float32)[64 * core:64 * (core + 1), :]
    wp = np.empty((128, T, 8, 64), dtype=np.float32)
    for t in range(T):
        for m in range(8):
            wp[:, t, m, :] = hW1[:, t * 1024 + m * 128: t * 1024 + (m + 1) * 128].T
    return _bf(wp)


def pack_hw2(hW2):
    """hW2 (112,512) -> [128, 4j, 112]: [ph*64+p, j, o] = hW2[o, (2j+ph)*64+p]."""
    hW2 = np.asarray(hW2, np.float32)
    wp = np.empty((128, 4, 112), dtype=np.float32)
    for ph in range(2):
        for j in range(4):
            wp[ph * 64:(ph + 1) * 64, j, :] = hW2[:, (2 * j + ph) * 64:(2 * j + ph + 1) * 64].T
    return _bf(wp)


# ----------------------------------------------------------------- program

def build_program(n_cores):
    nc = bacc.Bacc("TRN2", target_bir_lowering=False, debug=False,
                   num_devices=n_cores)
    core_ids = list(range(n_cores))
    use_cc = n_cores > 1

    ein = lambda name, shape, dt=BF: nc.dram_tensor(name, shape, dt, kind="ExternalInput")
    q_d = ein("qim", [108, FPC, 28, 58])
    w1p0_d = ein("w1p0", [108, 128])
    w1p1_d = ein("w1p1", [108, 128])
    id_d = ein("idf32", [128, 128], F32)
    w2A_d = ein("w2A", [128, 3, 128])
    w2B_d = ein("w2B", [64, 3, 128])
    w3_d = ein("w3p", [128, 9, 2, 128])
    w4_d = ein("w4p", [128, 9, 2, 4, 128])
    wi_d = ein("wip", [128, 4, 8, 128])
    ws_d = ein("wsp", [64, 8, 128])
    wr_d = ein("wrp", [128, 8, 8, 128])
    h1_d = ein("h1p", [128, T, 8, 64])
    h2_d = ein("h2p", [128, 4, 112])
    st_d = ein("stT", [14, T])
    swt_d = ein("swT", [14, 64])
    gb1_d = ein("gb1", [64, 2], F32)
    gb2_d = ein("gb2", [128, 2], F32)
    gb3_d = ein("gb3", [128, 4], F32)    # cols: g_h0, g_h1, b_h0, b_h1
    gb4_d = ein("gb4", [128, 8], F32)    # cols: g x4m then b x4m
    sb_d = ein("sbv", [64, 1], F32)
    bres_d = ein("bres", [128, 8], F32)
    hb1_d = ein("hb1c", [64, 1], F32)    # this core's 64-row slice of hb1
    hb2_d = ein("hb2p", [112, 1], F32)
    mask_d = ein("maskr", [64, 8], F32)   # ones in own rank's column
    maskb_d = ein("maskb", [64, 64], F32)  # [p, r*8+b]: 1 iff b == own rank

    out_d = nc.dram_tensor("out", [112, 1], F32, kind="ExternalOutput")
    if DEBUG_DUMPS:
        dbg_xin = nc.dram_tensor("dbg_xin", [128, T, 8], F32, kind="ExternalOutput")
        dbg_s = nc.dram_tensor("dbg_s", [128, T, 8], F32, kind="ExternalOutput")
        dbg_h = nc.dram_tensor("dbg_h", [64, 8], F32, kind="ExternalOutput")
        dbg_a2 = nc.dram_tensor("dbg_a2", [128, 30, 30], BF, kind="ExternalOutput")
        dbg_a3 = nc.dram_tensor("dbg_a3", [128, 16, 16], BF, kind="ExternalOutput")
        dbg_y4 = nc.dram_tensor("dbg_y4", [128, 4, 16], BF, kind="ExternalOutput")

    if use_cc:
        cc_shapes = [[64, 2], [128, 2], [128, 4], [128, 8], [64, 64], [64, 1]]
        cc_i = [nc.dram_tensor(f"cc{i}i", s, F32) for i, s in enumerate(cc_shapes)]
        cc_o = [nc.dram_tensor(f"cc{i}o", s, F32, addr_space="Shared")
                for i, s in enumerate(cc_shapes)]
        rg1_i = nc.dram_tensor("rg1i", [128, T // 2, 8], F32)
        rg1_o = nc.dram_tensor("rg1o", [n_cores, 128, T // 2, 8], F32,
                               addr_space="Shared")
        rg2_i = nc.dram_tensor("rg2i", [128, T // 2, 8], F32)
        rg2_o = nc.dram_tensor("rg2o", [n_cores, 128, T // 2, 8], F32,
                               addr_space="Shared")

    BTOT = n_cores
    CNT = [float(BTOT * NI * T * hw) for hw in (56 * 56, 14 * 14, 7 * 7, 4 * 4)]

    with tile.TileContext(nc) as tc:
        with (
            tc.tile_pool(name="wts", bufs=1) as wts,
            tc.tile_pool(name="acts", bufs=1) as acts,
            tc.tile_pool(name="qstream", bufs=3) as qstream,
            tc.tile_pool(name="ps", bufs=2, space="PSUM") as ps,
            tc.tile_pool(name="ev", bufs=2) as ev,
            tc.tile_pool(name="sq", bufs=1) as sqp,
            tc.tile_pool(name="misc", bufs=1) as misc,
        ):
            def pbank():
                return ps.tile([128, 4, 512], F32, tag="ps", name="psb")
            def load(dram, shape, dtype=BF):
                tl = wts.tile(shape, dtype, tag=dram.name + "_sb")
                nc.sync.dma_start(tl[:], dram[:])
                return tl

            # ---- collective warmup: tiny AllReduce right at kernel start so
            # the CC cores are hot by the time BN1 stats are ready.
            if use_cc:
                warm_sb = misc.tile([64, 1], F32, tag="warm_sb")
                nc.vector.memset(warm_sb[:], 0.0)
                nc.sync.dma_start(cc_i[5][:], warm_sb[:])
                nc.gpsimd.collective_compute(
                    "AllReduce", ALU.add, ins=[cc_i[5][:]], outs=[cc_o[5][:]],
                    replica_groups=[core_ids])
                warm_g = misc.tile([64, 1], F32, tag="warm_g")
                nc.sync.dma_start(warm_g[:], cc_o[5][:])

            w1p0 = load(w1p0_d, [108, 128])
            w1p1 = load(w1p1_d, [108, 128])
            idf = load(id_d, [128, 128], F32)
            w2A = load(w2A_d, [128, 3, 128])
            w2B = load(w2B_d, [64, 3, 128])
            w3s = load(w3_d, [128, 9, 2, 128])
            w4s = load(w4_d, [128, 9, 2, 4, 128])
            wis = load(wi_d, [128, 4, 8, 128])
            wss = load(ws_d, [64, 8, 128])
            wrs = load(wr_d, [128, 8, 8, 128])
            h1s = load(h1_d, [128, T, 8, 64])
            h2s = load(h2_d, [128, 4, 112])
            sts = load(st_d, [14, T])
            swts = load(swt_d, [14, 64])
            gb1 = load(gb1_d, [64, 2], F32)
            gb2 = load(gb2_d, [128, 2], F32)
            gb3 = load(gb3_d, [128, 4], F32)
            gb4 = load(gb4_d, [128, 8], F32)
            sbv = load(sb_d, [64, 1], F32)
            bres = load(bres_d, [128, 8], F32)
            hb1c = load(hb1_d, [64, 1], F32)
            hb2 = load(hb2_d, [112, 1], F32)
            maskr = load(mask_d, [64, 8], F32)
            maskb = load(maskb_d, [64, 64], F32)

            A2 = acts.tile([128, FPC, 30, 30], BF)
            A3 = acts.tile([128, FPC, 16, 16], BF)
            A4 = acts.tile([128, 2, FPC, 9, 9], BF)
            Y4 = acts.tile([128, 4, FPC, 16], BF)
            # zero only the pad positions each conv reads but never writes
            # (flat per-frame views keep every memset at <=3 total dims)
            A2f = A2.rearrange("p f y x -> p f (y x)")
            A3f = A3.rearrange("p f y x -> p f (y x)")
            A4f = A4.rearrange("p h f y x -> p (h f) (y x)")
            nc.vector.memset(A2f[0:64, :, 0:30], 0.0)        # row 0
            nc.vector.memset(A2f[0:64, :, 30:871:30], 0.0)   # col 0, rows 1..28
            nc.vector.memset(A2f[0:64, :, 59:900:30], 0.0)   # col 29 (dup-shift src)
            nc.vector.memset(A3f[:, :, 0:15], 0.0)           # row 0
            nc.vector.memset(A3f[:, :, 16:240:16], 0.0)      # col 0, rows 1..14
            nc.vector.memset(A4f[:, :, 0:9], 0.0)            # row 0
            nc.vector.memset(A4f[:, :, 72:81], 0.0)          # row 8
            nc.vector.memset(A4f[:, :, 9:72:9], 0.0)         # col 0, rows 1..7
            nc.vector.memset(A4f[:, :, 17:72:9], 0.0)        # col 8, rows 1..7

            s1acc = acts.tile([128, FPC * 2], F32)
            q1acc = acts.tile([128, FPC], F32)
            s2acc = acts.tile([128, 16], F32)
            q2acc = acts.tile([128, 16], F32)
            s3acc = acts.tile([128, 8], F32)
            q3acc = acts.tile([128, 8], F32)
            s4acc = acts.tile([128, 4], F32)
            q4acc = acts.tile([128, 4], F32)

            # ================ state embedding (own b): SE [64, 16] bf16
            # (independent of the CNN -- emit first so it fills startup gaps)
            pse0 = pbank()
            nc.tensor.matmul(pse0[0:64, 0, 0:T], swts[:], sts[:],
                             start=True, stop=True)
            SE = acts.tile([64, T], BF)
            nc.scalar.activation(SE[:], pse0[0:64, 0, 0:T], AF.Relu,
                                 bias=sbv[:, 0:1])

            # ================ conv1 + maxpool, frames in pairs
            for fp in range(FPC // 2):
                yf2 = ev.tile([128, 2, 2, 28, 28], BF, tag="yf2")
                for fi in range(2):
                    f = 2 * fp + fi
                    qf = qstream.tile([108, 28, 58], BF, tag="qf")
                    nc.sync.dma_start(qf[:], q_d[:, f, :, :])
                    pt = pbank()
                    for g in range(4):
                        ptv = pt[:, g, 0:392].rearrange("p (r x) -> p r x", r=7)
                        nc.tensor.matmul(ptv, w1p0[:], qf[:, 7 * g:7 * g + 7, 0:56],
                                         start=True, stop=False)
                        nc.tensor.matmul(ptv, w1p1[:], qf[:, 7 * g:7 * g + 7, 2:58],
                                         start=False, stop=True)
                    # copy out de-interleaved (even/odd x) + per-frame sums
                    for par in range(2):
                        nc.scalar.activation(
                            yf2[:, par, fi].rearrange("p (g r) u -> p g r u", g=4),
                            pt[:, :, par:392:2].rearrange("p g (r u) -> p g r u", r=7),
                            AF.Copy,
                            accum_out=s1acc[:, 2 * f + par:2 * f + par + 1])
                # x-direction pool for both frames at once (packed -> 4x STT)
                evm = yf2[:, 0].rearrange("p f y u -> p (f y) u")
                odm = yf2[:, 1].rearrange("p f y u -> p (f y) u")
                ax = ev.tile([128, 2, 28, 28], BF, tag="ax")
                axm = ax[:].rearrange("p f y u -> p (f y) u")
                nc.vector.scalar_tensor_tensor(
                    out=axm, in0=evm, scalar=1.0, in1=odm,
                    op0=ALU.mult, op1=ALU.max)
                nc.vector.scalar_tensor_tensor(
                    out=axm[:, :, 1:28], in0=axm[:, :, 1:28], scalar=1.0,
                    in1=odm[:, :, 0:27], op0=ALU.mult, op1=ALU.max)
                for fi in range(2):
                    f = 2 * fp + fi
                    # sum of squares (4x STT, dead scratch out)
                    sq1 = sqp.tile([128, 2, 784], BF, tag="sq1")
                    yfv = yf2[:, :, fi].rearrange("p par y u -> p par (y u)")
                    nc.vector.scalar_tensor_tensor(
                        out=sq1[:], in0=yfv, scalar=1.0, in1=yfv,
                        op0=ALU.mult, op1=ALU.mult,
                        accum_out=q1acc[:, f:f + 1])
                    # y-direction pool: rows y=2r+ry live as (partition ry, r)
                    pxb = ev.tile([64, 28, 28], BF, tag="pxb")
                    nc.gpsimd.tensor_scalar_add(pxb[:], ax[64:128, fi], 0.0)
                    nc.vector.scalar_tensor_tensor(
                        out=A2[0:64, f, 1:29, 1:29], in0=ax[0:64, fi], scalar=1.0,
                        in1=pxb[:], op0=ALU.mult, op1=ALU.max)
                    nc.vector.scalar_tensor_tensor(
                        out=A2[0:64, f, 2:29, 1:29], in0=A2[0:64, f, 2:29, 1:29],
                        scalar=1.0, in1=pxb[:, 0:27, :], op0=ALU.mult, op1=ALU.max)

            # ---------------- BN helpers
            def bn_finalize(s_red, q_red, gb, cnt, P, K, lname):
                m = misc.tile([P, K], F32, tag=lname + "_m")
                nc.vector.tensor_scalar_mul(m[:], s_red[:], 1.0 / cnt)
                msq = misc.tile([P, K], F32, tag=lname + "_msq")
                nc.vector.tensor_mul(msq[:], m[:], m[:])
                v = misc.tile([P, K], F32, tag=lname + "_v")
                nc.vector.scalar_tensor_tensor(out=v[:], in0=q_red[:], scalar=1.0 / cnt,
                                               in1=msq[:], op0=ALU.mult,
                                               op1=ALU.subtract)
                ve = misc.tile([P, K], F32, tag=lname + "_ve")
                nc.vector.tensor_scalar_add(ve[:], v[:], float(BN_EPS))
                rc = misc.tile([P, K], F32, tag=lname + "_rc")
                nc.vector.reciprocal(rc[:], ve[:])
                rinv = misc.tile([P, K], F32, tag=lname + "_rinv")
                nc.scalar.activation(rinv[:], rc[:], AF.Sqrt)
                a = misc.tile([P, K], F32, tag=lname + "_a")
                nc.vector.tensor_mul(a[:], rinv[:], gb[:, 0:K])
                tmv = misc.tile([P, K], F32, tag=lname + "_tmv")
                nc.vector.tensor_mul(tmv[:], m[:], a[:])
                b = misc.tile([P, K], F32, tag=lname + "_b")
                nc.vector.tensor_sub(b[:], gb[:, K:2 * K], tmv[:])
                return a, b

            def allreduce(idx, src, P, K, lname):
                if not use_cc:
                    return src
                nc.sync.dma_start(cc_i[idx][:], src[:])
                nc.gpsimd.collective_compute(
                    "AllReduce", ALU.add, ins=[cc_i[idx][:]], outs=[cc_o[idx][:]],
                    replica_groups=[core_ids])
                g = misc.tile([P, K], F32, tag=lname + "_g")
                nc.sync.dma_start(g[:], cc_o[idx][:])
                return g

            # ---------------- BN1 -> A2
            s1r = misc.tile([128, 1], F32, tag="s1r")
            nc.vector.reduce_sum(s1r[:], s1acc[:], axis=AX.X)
            q1r = misc.tile([128, 1], F32, tag="q1r")
            nc.vector.reduce_sum(q1r[:], q1acc[:], axis=AX.X)
            st1 = misc.tile([64, 2], F32, tag="st1")
            s1hi = misc.tile([64, 2], F32, tag="s1hi")
            nc.vector.tensor_copy(s1hi[:, 0:1], s1r[64:128, :])
            nc.vector.tensor_copy(s1hi[:, 1:2], q1r[64:128, :])
            nc.vector.tensor_add(st1[:, 0:1], s1r[0:64, :], s1hi[:, 0:1])
            nc.vector.tensor_add(st1[:, 1:2], q1r[0:64, :], s1hi[:, 1:2])
            if use_cc:
                # fold 0*warm_g into the stats so the warmup collective is
                # not dead code (also serializes BN1 after the warmup).
                nc.vector.scalar_tensor_tensor(
                    out=st1[:, 0:1], in0=warm_g[:], scalar=0.0,
                    in1=st1[:, 0:1], op0=ALU.mult, op1=ALU.add)
            g1 = allreduce(0, st1, 64, 2, "l1")
            a1, b1 = bn_finalize(g1[:, 0:1], g1[:, 1:2], gb1, CNT[0], 64, 1, "l1")

            # ================ BN1-relu + x-shifted dup + conv2, interleaved
            # per 8-frame block so conv2 starts as soon as block 0 is ready.
            for blk in range(4):
                f0 = blk * 8
                nc.scalar.activation(A2[0:64, f0:f0 + 8, 1:29, 1:29],
                                     A2[0:64, f0:f0 + 8, 1:29, 1:29],
                                     AF.Relu, bias=b1[:, 0:1], scale=a1[:, 0:1])
                # dup: one flat packed copy (col 29 of src is zeroed; wrapped
                # cells land where conv2 never reads)
                nc.vector.tensor_copy(A2f[64:128, f0:f0 + 8, 0:870],
                                      A2f[0:64, f0:f0 + 8, 1:871])
                for g in range(blk * 4, blk * 4 + 4):
                    fg = g * 2
                    pt = pbank()
                    ptv = pt[:, 0, 0:392].rearrange("p (f y x) -> p f y x",
                                                    f=2, y=14)
                    for ky in range(3):
                        nc.tensor.matmul(ptv, w2A[:, ky, :],
                                         A2[:, fg:fg + 2, ky:ky + 28:2, 0:28:2],
                                         start=(ky == 0), stop=False)
                        nc.tensor.matmul(ptv, w2B[:, ky, :],
                                         A2[0:64, fg:fg + 2, ky:ky + 28:2, 2:30:2],
                                         start=False, stop=(ky == 2))
                    nc.scalar.activation(A3[:, fg:fg + 2, 1:15, 1:15], ptv,
                                         AF.Copy, accum_out=s2acc[:, g:g + 1])
                    sq2 = sqp.tile([128, 2, 14, 14], BF, tag="sq2")
                    nc.scalar.activation(sq2[:], A3[:, fg:fg + 2, 1:15, 1:15],
                                         AF.Square, accum_out=q2acc[:, g:g + 1])

            st2 = misc.tile([128, 2], F32, tag="st2")
            nc.vector.reduce_sum(st2[:, 0:1], s2acc[:], axis=AX.X)
            nc.vector.reduce_sum(st2[:, 1:2], q2acc[:], axis=AX.X)
            g2 = allreduce(1, st2, 128, 2, "l2")
            a2, b2 = bn_finalize(g2[:, 0:1], g2[:, 1:2], gb2, CNT[1], 128, 1, "l2")

            # ================ BN2-relu + conv3 interleaved per 8-frame block
            for g in range(4):
                f0 = g * 8
                nc.scalar.activation(A3[:, f0:f0 + 8, 1:15, 1:15],
                                     A3[:, f0:f0 + 8, 1:15, 1:15],
                                     AF.Relu, bias=b2[:, 0:1], scale=a2[:, 0:1])
                for m in range(2):
                    pt = pbank()
                    ptv = pt[:, 0, 0:392].rearrange("p (f y x) -> p f y x",
                                                    f=8, y=7)
                    for tap in range(9):
                        ky, kx = tap // 3, tap % 3
                        nc.tensor.matmul(ptv, w3s[:, tap, m, :],
                                         A3[:, f0:f0 + 8, ky:ky + 14:2, kx:kx + 14:2],
                                         start=(tap == 0), stop=(tap == 8))
                    ci = m * 4 + g
                    nc.scalar.activation(A4[:, m, f0:f0 + 8, 1:8, 1:8], ptv,
                                         AF.Copy, accum_out=s3acc[:, ci:ci + 1])
                    sq3 = sqp.tile([128, 8, 7, 7], BF, tag="sq3")
                    nc.scalar.activation(sq3[:], A4[:, m, f0:f0 + 8, 1:8, 1:8],
                                         AF.Square, accum_out=q3acc[:, ci:ci + 1])

            st3 = misc.tile([128, 4], F32, tag="st3")
            for m in range(2):
                nc.vector.reduce_sum(st3[:, m:m + 1], s3acc[:, m * 4:(m + 1) * 4],
                                     axis=AX.X)
                nc.vector.reduce_sum(st3[:, 2 + m:3 + m], q3acc[:, m * 4:(m + 1) * 4],
                                     axis=AX.X)
            g3 = allreduce(2, st3, 128, 4, "l3")
            a3, b3 = bn_finalize(g3[:, 0:2], g3[:, 2:4], gb3, CNT[2], 128, 2, "l3")
            for h in range(2):
                nc.scalar.activation(
                    A4[:, h, :, 1:8, 1:8], A4[:, h, :, 1:8, 1:8],
                    AF.Relu, bias=b3[:, h:h + 1], scale=a3[:, h:h + 1])

            # ================ conv4 (h-outer so m=0 can start after relu h=0)
            for m in range(4):
                pt = pbank()
                ptv = pt[:, 0, :].rearrange("p (f y x) -> p f y x", f=FPC, y=4)
                first = True
                for h in range(2):
                    for tap in range(9):
                        ky, kx = tap // 3, tap % 3
                        nc.tensor.matmul(ptv, w4s[:, tap, h, m, :],
                                         A4[:, h, :, ky:ky + 7:2, kx:kx + 7:2],
                                         start=first, stop=(tap == 8 and h == 1))
                        first = False
                nc.scalar.activation(
                    Y4[:, m, :, :].rearrange("p f (y x) -> p f y x", y=4),
                    ptv, AF.Copy, accum_out=s4acc[:, m:m + 1])
                sq4 = sqp.tile([128, FPC, 16], BF, tag="sq4")
                nc.vector.scalar_tensor_tensor(
                    out=sq4[:], in0=Y4[:, m, :, :], scalar=1.0, in1=Y4[:, m, :, :],
                    op0=ALU.mult, op1=ALU.mult,
                    accum_out=q4acc[:, m:m + 1])

            st4 = misc.tile([128, 8], F32, tag="st4")
            nc.vector.tensor_copy(st4[:, 0:4], s4acc[:])
            nc.vector.tensor_copy(st4[:, 4:8], q4acc[:])
            g4 = allreduce(3, st4, 128, 8, "l4")
            a4, b4 = bn_finalize(g4[:, 0:4], g4[:, 4:8], gb4, CNT[3], 128, 4, "l4")

            # BN4+relu -> avgpool(16px) -> image-mean -> feats [128,(4m,16t)]
            S4 = acts.tile([128, 4, FPC], F32)
            for m in range(4):
                r4m = ev.tile([128, FPC, 16], F32, tag="r4m")
                nc.scalar.activation(r4m[:], Y4[:, m, :, :], AF.Relu,
                                     bias=b4[:, m:m + 1], scale=a4[:, m:m + 1])
                nc.vector.reduce_sum(S4[:, m, :], r4m[:], axis=AX.X)
            fsum = acts.tile([128, 4, T], F32)
            nc.vector.tensor_add(fsum[:], S4[:, :, 0:T], S4[:, :, T:FPC])
            FIb = acts.tile([128, 4, T], BF)
            nc.vector.tensor_scalar_mul(FIb[:], fsum[:], 1.0 / 32.0)

            # ================ xin[p, t, m] = (Win @ step_inputs)_t + b_res
            xin = acts.tile([128, T, 8], F32)
            for m in range(8):
                pxt = pbank()
                px2 = pxt[:, 0, 0:T]
                for ki in range(4):
                    nc.tensor.matmul(px2, wis[:, ki, m, :], FIb[:, ki, :],
                                     start=(ki == 0), stop=False)
                nc.tensor.matmul(px2, wss[:, m, :], SE[:], start=False, stop=True)
                nc.scalar.activation(xin[:, :, m], px2, AF.Identity,
                                     bias=bres[:, m:m + 1])

            # ================ reservoir scan (own b), s = 2r form:
            # s_t = 0.5*s_{t-1} + tanh(xin_t + (0.5*Wres) s_{t-1})
            # xin is injected into PSUM via an fp32 identity matmul so the
            # scalar engine can tanh straight out of the bank.
            s_hist = acts.tile([128, T, 8], F32)
            sb16 = acts.tile([128, T, 8], BF)
            szero = acts.tile([128, 8], F32)
            szero16 = acts.tile([128, 8], BF)
            nc.vector.memset(szero[:], 0.0)
            nc.vector.memset(szero16[:], 0.0)
            for t in range(T):
                prt = pbank()
                nc.tensor.matmul(prt[:, 0, 0:8], idf[:], xin[:, t, :],
                                 start=True, stop=False, skip_group_check=True)
                for m in range(8):
                    for k in range(8):
                        rhs = (szero16[:, k:k + 1] if t == 0
                               else sb16[:, t - 1, k:k + 1])
                        nc.tensor.matmul(prt[:, 0, m:m + 1], wrs[:, k, m, :],
                                         rhs, start=False,
                                         stop=(m == 7 and k == 7),
                                         skip_group_check=True)
                th = ev.tile([128, 8], F32, tag="scan_th")
                nc.scalar.activation(th[:], prt[:, 0, 0:8], AF.Tanh)
                sprev = szero[:] if t == 0 else s_hist[:, t - 1, :]
                nc.vector.scalar_tensor_tensor(
                    out=s_hist[:, t, :], in0=sprev, scalar=0.5, in1=th[:],
                    op0=ALU.mult, op1=ALU.add)
                nc.vector.tensor_copy(sb16[:, t, :], s_hist[:, t, :])
                if t == T // 2 - 1:
                    # first-half AllGather overlaps the rest of the scan
                    nc.sync.dma_start(rg1_i[:], s_hist[:, 0:T // 2, :])
                    nc.gpsimd.collective_compute(
                        "AllGather", ALU.bypass, ins=[rg1_i[:]],
                        outs=[rg1_o[:]], replica_groups=[core_ids])

            # ================ head. hW1 is sharded by output rows; every core
            # computes its 64-row slice for ALL batches from the AllGathered
            # reservoir histories, then a masked AllReduce redistributes each
            # batch's full H.
            nc.sync.dma_start(rg2_i[:], s_hist[:, T // 2:T, :])
            nc.gpsimd.collective_compute(
                "AllGather", ALU.bypass, ins=[rg2_i[:]], outs=[rg2_o[:]],
                replica_groups=[core_ids])
            Rf = acts.tile([128, n_cores, T, 8], F32)
            Rb16 = acts.tile([128, n_cores, T, 8], BF)
            nc.sync.dma_start(Rf[:, :, 0:T // 2, :],
                              rg1_o[:].rearrange("b p t m -> p b t m"))
            nc.vector.tensor_copy(Rb16[:, :, 0:T // 2, :], Rf[:, :, 0:T // 2, :])
            pht = pbank()
            ph = pht[0:64, 0, 0:8]
            first = True
            for t in range(T // 2):
                for m in range(8):
                    nc.tensor.matmul(ph, h1s[:, t, m, :], Rb16[:, :, t, m],
                                     start=first, stop=False,
                                     skip_group_check=True)
                    first = False
            nc.sync.dma_start(Rf[:, :, T // 2:T, :],
                              rg2_o[:].rearrange("b p t m -> p b t m"))
            nc.vector.tensor_copy(Rb16[:, :, T // 2:T, :], Rf[:, :, T // 2:T, :])
            for t in range(T // 2, T):
                for m in range(8):
                    nc.tensor.matmul(ph, h1s[:, t, m, :], Rb16[:, :, t, m],
                                     start=False, stop=(t == T - 1 and m == 7),
                                     skip_group_check=True)
            hcs = misc.tile([64, 8], F32, tag="hcs")
            nc.scalar.activation(hcs[:], ph, AF.Identity, bias=hb1c[:, 0:1])
            # buf[p, r*8+b] = hcs[p, b] iff r == own rank, else 0
            hbuf = misc.tile([64, 64], F32, tag="hbuf")
            for r in range(n_cores):
                nc.vector.scalar_tensor_tensor(
                    out=hbuf[:, r * 8:(r + 1) * 8], in0=hcs[:],
                    scalar=maskr[:, r:r + 1],
                    in1=szero[0:64, 0:8], op0=ALU.mult, op1=ALU.add)
            gH = allreduce(4, hbuf, 64, 64, "lH")
            # select own batch's column: msel[p, r*8+b] = gH * (b == rank)
            msel = misc.tile([64, 64], F32, tag="msel")
            nc.vector.tensor_mul(msel[:], gH[:], maskb[:])
            Hsel = misc.tile([64, 8], F32, tag="Hsel")
            nc.vector.reduce_sum(
                Hsel[:], msel[:].rearrange("p (r b) -> p r b", b=8), axis=AX.X)
            H64 = misc.tile([64, 8], BF, tag="H64")
            nc.scalar.activation(H64[:], Hsel[:], AF.Relu)
            H128 = misc.tile([128, 4], BF, tag="H128")
            nc.vector.tensor_copy(H128[0:64, :], H64[:, 0:8:2])
            nc.vector.tensor_copy(H128[64:128, :], H64[:, 1:8:2])

            pot = pbank()
            po = pot[0:112, 0, 0:1]
            for j in range(4):
                nc.tensor.matmul(po, h2s[:, j, :], H128[:, j:j + 1],
                                 start=(j == 0), stop=(j == 3))
            OutS = acts.tile([112, 1], F32)
            nc.scalar.activation(OutS[:], po, AF.Identity, bias=hb2[:, 0:1])
            nc.sync.dma_start(out_d[:], OutS[:])
            if DEBUG_DUMPS:
                nc.sync.dma_start(dbg_xin[:], xin[:])
                nc.sync.dma_start(dbg_s[:], s_hist[:])
                nc.sync.dma_start(dbg_h[:], Hsel[:])
                nc.sync.dma_start(dbg_a2[:], A2[:, 0, :, :])
                nc.sync.dma_start(dbg_a3[:], A3[:, 0, :, :])
                nc.sync.dma_start(dbg_y4[:], Y4[:, :, 0, :].rearrange("p m x -> p m x"))

    nc.compile()
    return nc


# ----------------------------------------------------------------- host driver

_CACHE = {}


def make_in_map(inputs, core):
    b = core
    imgs = np.asarray(inputs["images_seq"], np.float32)[b].reshape(FPC, 3, 112, 112)
    wA, wB = pack_w2(inputs["w2"])
    wi, ws = pack_win(inputs["Win"])
    f32 = lambda x: np.asarray(x, np.float32)
    mask = np.zeros((64, 8), dtype=np.float32)
    mask[:, core] = 1.0
    maskb = np.zeros((64, 8, 8), dtype=np.float32)
    maskb[:, :, core] = 1.0
    w1p0, w1p1 = pack_w1_d4(inputs["w1"])
    d = {
        "qim": conv1_q(imgs),
        "w1p0": w1p0, "w1p1": w1p1,
        "idf32": np.eye(128, dtype=np.float32),
        "w2A": wA, "w2B": wB,
        "w3p": pack_w3(inputs["w3"]),
        "w4p": pack_w4(inputs["w4"]),
        "wip": wi, "wsp": ws,
        "wrp": pack_wres(inputs["Wres"]),
        "h1p": pack_hw1_slice(inputs["hW1"], core),
        "h2p": pack_hw2(inputs["hW2"]),
        "stT": _bf(f32(inputs["state_seq"])[b].T),
        "swT": _bf(f32(inputs["sW"]).T),
        "gb1": np.stack([f32(inputs["g1"]), f32(inputs["be1"])], axis=1),
        "gb2": np.stack([f32(inputs["g2"]), f32(inputs["be2"])], axis=1),
        "gb3": np.concatenate([f32(inputs["g3"]).reshape(2, 128).T,
                               f32(inputs["be3"]).reshape(2, 128).T], axis=1),
        "gb4": np.concatenate([f32(inputs["g4"]).reshape(4, 128).T,
                               f32(inputs["be4"]).reshape(4, 128).T], axis=1),
        "sbv": f32(inputs["sb"]).reshape(64, 1),
        "bres": f32(inputs["b_res"]).reshape(8, 128).T.copy(),
        "hb1c": f32(inputs["hb1"])[64 * core:64 * (core + 1)].reshape(64, 1),
        "hb2p": f32(inputs["hb2"]).reshape(112, 1),
        "maskr": mask,
        "maskb": maskb.reshape(64, 64),
    }
    return d


def run(inputs, n_cores=8, **kw):
    core_ids = list(range(n_cores))
    if n_cores not in _CACHE:
        _CACHE[n_cores] = build_program(n_cores)
    nc = _CACHE[n_cores]
    in_maps = [make_in_map(inputs, c) for c in core_ids]
    res = run_bass_kernel_spmd(nc, in_maps, core_ids, **kw)
    rows = [np.asarray(res.results[c]["out"], np.float32).reshape(112)
            for c in core_ids]
    return np.stack(rows, axis=0), res


def kernel(**inputs):
    out, _ = run(inputs, n_cores=8)
    return out.reshape(8, 8, 14)


# revision 42
# speedup vs baseline: 1.5924x; 1.5924x over previous
"""CnnReservoirPolicy Trainium2 kernel (8-core SPMD).

Sharding: batch element b -> core b (32 CNN frames/core). BatchNorm batch
statistics (training mode, over the full 256-frame batch) are the only
cross-core CNN dependency: 4 tiny AllReduces. The readout head hW1 is
sharded by output rows (64 rows/core); each core computes its own batch
element's partial H, masked into a [64, 8] rank-column buffer and
AllReduced so every core gets the full 512-dim hidden vector.

conv1 (7x7 s2) runs as a GEMM with M = 64ch x 2 output-row-parity and
K = 189 = (3c x 9yrel x 7kx); its im2col tensor is built host-side.
conv2 packs kx-pairs into K=128 via an x-shifted duplicate of its input.
All matmuls bf16 with fp32 PSUM accumulation. maxpool runs on raw conv1
outputs (commutes with the positive-scale BN affine + relu).

The reservoir scan runs in s = 2r form (0.5*Wres folded host-side, and
0.5*hW1 for the readout) so the update is one scalar_tensor_tensor:
s_t = 0.5*s_{t-1} + tanh(xin_t + (0.5*Wres) s_{t-1}).
"""
import os
import sys

for _p in ("/opt/trn_rl_repo", os.path.expanduser("~/.axon_site/_ro/trn_rl_repo")):
    if os.path.isdir(_p) and _p not in sys.path:
        sys.path.insert(0, _p)

import numpy as np
import ml_dtypes

import concourse.bass as bass
import concourse.mybir as mybir
import concourse.tile as tile
from concourse import bacc
from concourse.bass_utils import run_bass_kernel_spmd

BF16 = ml_dtypes.bfloat16
F32 = mybir.dt.float32
BF = mybir.dt.bfloat16
AX = mybir.AxisListType
ALU = mybir.AluOpType
AF = mybir.ActivationFunctionType

ALPHA = 0.5
BN_EPS = 1e-5
DEBUG_DUMPS = False
NI, T = 2, 16          # num_images, n_obs_steps
FPC = NI * T           # frames per core = 32
K1 = 189               # conv1 GEMM contraction (3c x 9yrel x 7kx)
K1A = 128
K1B = K1 - K1A         # 61


# ----------------------------------------------------------------- host packing

def _bf(x):
    return np.ascontiguousarray(np.asarray(x, np.float32)).astype(BF16)


def conv1_q(imgs):
    """imgs (F,3,112,112) f32 -> Q (108, F, 28, 58) bf16.

    Q[((q*2+par)*3+c)*9+yr, f, r, u] = pad3(imgs)[f, c, 4r+yr, 2(u+q)+par].
    Pass p reads the x-window [2p : 2p+56] and contributes tap kx=2q+par+4p;
    output row oy = 2r+ry pairs with weight column block ry.
    """
    F = imgs.shape[0]
    pb = np.pad(np.asarray(imgs, np.float32),
                ((0, 0), (0, 0), (3, 3), (3, 3))).astype(BF16)
    Q = np.empty((2, 2, 3, 9, F, 28, 58), dtype=BF16)
    for q in range(2):
        for par in range(2):
            x0 = 2 * q + par
            for yr in range(9):
                Q[q, par, :, yr] = pb[:, :, yr:yr + 112:4,
                                      x0:x0 + 116:2].transpose(1, 0, 2, 3)
    return Q.reshape(108, F, 28, 58)


def pack_w1_d4(w1):
    """w1 (64,3,7,7) -> 2 passes of [108, 128]; col = ry*64+co."""
    w1 = np.asarray(w1, np.float32)
    W = np.zeros((2, 2, 2, 3, 9, 128), dtype=np.float32)
    for p_ in range(2):
        for q in range(2):
            for par in range(2):
                kx = 2 * q + par + 4 * p_
                if kx > 6:
                    continue
                for ry in range(2):
                    for ky in range(7):
                        yr = 2 * ry + ky
                        W[p_, q, par, :, yr, ry * 64:(ry + 1) * 64] = \
                            w1[:, :, ky, kx].T
    Wf = W.reshape(2, 108, 128)
    return _bf(Wf[0]), _bf(Wf[1])


def pack_w2(w2):
    """w2 (128,64,3,3) -> A [128=(d,ci),3ky,128co], B [64ci,3ky,128co]."""
    w2 = np.asarray(w2, np.float32)
    wA = np.zeros((128, 3, 128), dtype=np.float32)
    wB = np.zeros((64, 3, 128), dtype=np.float32)
    for ky in range(3):
        for d in range(2):
            wA[d * 64:(d + 1) * 64, ky, :] = w2[:, :, ky, d].T
        wB[:, ky, :] = w2[:, :, ky, 2].T
    return _bf(wA), _bf(wB)


def pack_w3(w3):
    """w3 (256,128,3,3) -> [128ci, 9tap, 2m, 128co]."""
    w3 = np.asarray(w3, np.float32)
    wp = np.empty((128, 9, 2, 128), dtype=np.float32)
    for tap in range(9):
        ky, kx = tap // 3, tap % 3
        for m in range(2):
            wp[:, tap, m, :] = w3[m * 128:(m + 1) * 128, :, ky, kx].T
    return _bf(wp)


def pack_w4(w4):
    """w4 (512,256,3,3) -> [128ci, 9tap, 2h, 4m, 128co]."""
    w4 = np.asarray(w4, np.float32)
    wp = np.empty((128, 9, 2, 4, 128), dtype=np.float32)
    for tap in range(9):
        ky, kx = tap // 3, tap % 3
        for h in range(2):
            for m in range(4):
                wp[:, tap, h, m, :] = \
                    w4[m * 128:(m + 1) * 128, h * 128:(h + 1) * 128, ky, kx].T
    return _bf(wp)


def pack_win(Win):
    """Win (1024,576) -> img [128, 4ki, 8m, 128], state [64, 8m, 128]."""
    Win = np.asarray(Win, np.float32)
    wi = np.empty((128, 4, 8, 128), dtype=np.float32)
    ws = np.empty((64, 8, 128), dtype=np.float32)
    for m in range(8):
        for ki in range(4):
            wi[:, ki, m, :] = Win[m * 128:(m + 1) * 128, ki * 128:(ki + 1) * 128].T
        ws[:, m, :] = Win[m * 128:(m + 1) * 128, 512:].T
    return _bf(wi), _bf(ws)


def pack_wres(Wres):
    """0.5*Wres (1024,1024) -> [128, 8k, 8m, 128]: [p,k,m,q] = W[m*128+q, k*128+p]."""
    Wres = 0.5 * np.asarray(Wres, np.float32)
    wp = np.empty((128, 8, 8, 128), dtype=np.float32)
    for k in range(8):
        for m in range(8):
            wp[:, k, m, :] = Wres[m * 128:(m + 1) * 128, k * 128:(k + 1) * 128].T
    return _bf(wp)


def pack_hw1_slice(hW1, core):
    """0.5*hW1 rows [64c:64c+64) -> [128p, 16t, 8m, 64j]:
    [p,t,m,j] = 0.5*hW1[64c+j, t*1024 + m*128 + p]."""
    hW1 = 0.5 * np.asarray(hW1, np.float32)[64 * core:64 * (core + 1), :]
    wp = np.empty((128, T, 8, 64), dtype=np.float32)
    for t in range(T):
        for m in range(8):
            wp[:, t, m, :] = hW1[:, t * 1024 + m * 128: t * 1024 + (m + 1) * 128].T
    return _bf(wp)


def pack_hw2(hW2):
    """hW2 (112,512) -> [128, 4j, 112]: [ph*64+p, j, o] = hW2[o, (2j+ph)*64+p]."""
    hW2 = np.asarray(hW2, np.float32)
    wp = np.empty((128, 4, 112), dtype=np.float32)
    for ph in range(2):
        for j in range(4):
            wp[ph * 64:(ph + 1) * 64, j, :] = hW2[:, (2 * j + ph) * 64:(2 * j + ph + 1) * 64].T
    return _bf(wp)


# ----------------------------------------------------------------- program

def build_program(n_cores):
    nc = bacc.Bacc("TRN2", target_bir_lowering=False, debug=False,
                   num_devices=n_cores)
    core_ids = list(range(n_cores))
    use_cc = n_cores > 1

    ein = lambda name, shape, dt=BF: nc.dram_tensor(name, shape, dt, kind="ExternalInput")
    q_d = ein("qim", [108, FPC, 28, 58])
    w1p0_d = ein("w1p0", [108, 128])
    w1p1_d = ein("w1p1", [108, 128])
    id_d = ein("idf32", [128, 128], F32)
    w2A_d = ein("w2A", [128, 3, 128])
    w2B_d = ein("w2B", [64, 3, 128])
    w3_d = ein("w3p", [128, 9, 2, 128])
    w4_d = ein("w4p", [128, 9, 2, 4, 128])
    wi_d = ein("wip", [128, 4, 8, 128])
    ws_d = ein("wsp", [64, 8, 128])
    wr_d = ein("wrp", [128, 8, 8, 128])
    h1_d = ein("h1p", [128, T, 8, 64])
    h2_d = ein("h2p", [128, 4, 112])
    st_d = ein("stT", [14, T])
    swt_d = ein("swT", [14, 64])
    gb1_d = ein("gb1", [64, 2], F32)
    gb2_d = ein("gb2", [128, 2], F32)
    gb3_d = ein("gb3", [128, 4], F32)    # cols: g_h0, g_h1, b_h0, b_h1
    gb4_d = ein("gb4", [128, 8], F32)    # cols: g x4m then b x4m
    sb_d = ein("sbv", [64, 1], F32)
    bres_d = ein("bres", [128, 8], F32)
    hb1_d = ein("hb1c", [64, 1], F32)    # this core's 64-row slice of hb1
    hb2_d = ein("hb2p", [112, 1], F32)
    mask_d = ein("maskr", [64, 8], F32)   # ones in own rank's column
    maskb_d = ein("maskb", [64, 64], F32)  # [p, r*8+b]: 1 iff b == own rank

    out_d = nc.dram_tensor("out", [112, 1], F32, kind="ExternalOutput")
    if DEBUG_DUMPS:
        dbg_xin = nc.dram_tensor("dbg_xin", [128, T, 8], F32, kind="ExternalOutput")
        dbg_s = nc.dram_tensor("dbg_s", [128, T, 8], F32, kind="ExternalOutput")
        dbg_h = nc.dram_tensor("dbg_h", [64, 8], F32, kind="ExternalOutput")
        dbg_a2 = nc.dram_tensor("dbg_a2", [128, 30, 30], BF, kind="ExternalOutput")
        dbg_a3 = nc.dram_tensor("dbg_a3", [128, 16, 16], BF, kind="ExternalOutput")
        dbg_y4 = nc.dram_tensor("dbg_y4", [128, 4, 16], BF, kind="ExternalOutput")

    if use_cc:
        cc_shapes = [[64, 2], [128, 2], [128, 4], [128, 8], [64, 64], [64, 1]]
        cc_i = [nc.dram_tensor(f"cc{i}i", s, F32) for i, s in enumerate(cc_shapes)]
        cc_o = [nc.dram_tensor(f"cc{i}o", s, F32, addr_space="Shared")
                for i, s in enumerate(cc_shapes)]
        rg1_i = nc.dram_tensor("rg1i", [128, T // 2, 8], F32)
        rg1_o = nc.dram_tensor("rg1o", [n_cores, 128, T // 2, 8], F32,
                               addr_space="Shared")
        rg2_i = nc.dram_tensor("rg2i", [128, T // 2, 8], F32)
        rg2_o = nc.dram_tensor("rg2o", [n_cores, 128, T // 2, 8], F32,
                               addr_space="Shared")

    BTOT = n_cores
    CNT = [float(BTOT * NI * T * hw) for hw in (56 * 56, 14 * 14, 7 * 7, 4 * 4)]

    with tile.TileContext(nc) as tc:
        with (
            tc.tile_pool(name="wts", bufs=1) as wts,
            tc.tile_pool(name="acts", bufs=1) as acts,
            tc.tile_pool(name="qstream", bufs=3) as qstream,
            tc.tile_pool(name="ps", bufs=2, space="PSUM") as ps,
            tc.tile_pool(name="ev", bufs=2) as ev,
            tc.tile_pool(name="sq", bufs=1) as sqp,
            tc.tile_pool(name="misc", bufs=1) as misc,
        ):
            def pbank():
                return ps.tile([128, 4, 512], F32, tag="ps", name="psb")
            def load(dram, shape, dtype=BF):
                tl = wts.tile(shape, dtype, tag=dram.name + "_sb")
                nc.sync.dma_start(tl[:], dram[:])
                return tl

            # ---- collective warmup: tiny AllReduce right at kernel start so
            # the CC cores are hot by the time BN1 stats are ready.
            if use_cc:
                warm_sb = misc.tile([64, 1], F32, tag="warm_sb")
                nc.vector.memset(warm_sb[:], 0.0)
                nc.sync.dma_start(cc_i[5][:], warm_sb[:])
                nc.gpsimd.collective_compute(
                    "AllReduce", ALU.add, ins=[cc_i[5][:]], outs=[cc_o[5][:]],
                    replica_groups=[core_ids])
                warm_g = misc.tile([64, 1], F32, tag="warm_g")
                nc.sync.dma_start(warm_g[:], cc_o[5][:])

            w1p0 = load(w1p0_d, [108, 128])
            w1p1 = load(w1p1_d, [108, 128])
            idf = load(id_d, [128, 128], F32)
            w2A = load(w2A_d, [128, 3, 128])
            w2B = load(w2B_d, [64, 3, 128])
            w3s = load(w3_d, [128, 9, 2, 128])
            w4s = load(w4_d, [128, 9, 2, 4, 128])
            wis = load(wi_d, [128, 4, 8, 128])
            wss = load(ws_d, [64, 8, 128])
            wrs = load(wr_d, [128, 8, 8, 128])
            h1s = load(h1_d, [128, T, 8, 64])
            h2s = load(h2_d, [128, 4, 112])
            sts = load(st_d, [14, T])
            swts = load(swt_d, [14, 64])
            gb1 = load(gb1_d, [64, 2], F32)
            gb2 = load(gb2_d, [128, 2], F32)
            gb3 = load(gb3_d, [128, 4], F32)
            gb4 = load(gb4_d, [128, 8], F32)
            sbv = load(sb_d, [64, 1], F32)
            bres = load(bres_d, [128, 8], F32)
            hb1c = load(hb1_d, [64, 1], F32)
            hb2 = load(hb2_d, [112, 1], F32)
            maskr = load(mask_d, [64, 8], F32)
            maskb = load(maskb_d, [64, 64], F32)

            A2 = acts.tile([128, FPC, 30, 30], BF)
            A3 = acts.tile([128, FPC, 16, 16], BF)
            A4 = acts.tile([128, 2, FPC, 9, 9], BF)
            Y4 = acts.tile([128, 4, FPC, 16], BF)
            # zero only the pad positions each conv reads but never writes
            # (flat per-frame views keep every memset at <=3 total dims)
            A2f = A2.rearrange("p f y x -> p f (y x)")
            A3f = A3.rearrange("p f y x -> p f (y x)")
            A4f = A4.rearrange("p h f y x -> p (h f) (y x)")
            nc.vector.memset(A2f[0:64, :, 0:30], 0.0)        # row 0
            nc.vector.memset(A2f[0:64, :, 30:871:30], 0.0)   # col 0, rows 1..28
            nc.vector.memset(A2f[0:64, :, 59:900:30], 0.0)   # col 29 (dup-shift src)
            nc.vector.memset(A3f[:, :, 0:15], 0.0)           # row 0
            nc.vector.memset(A3f[:, :, 16:240:16], 0.0)      # col 0, rows 1..14
            nc.vector.memset(A4f[:, :, 0:9], 0.0)            # row 0
            nc.vector.memset(A4f[:, :, 72:81], 0.0)          # row 8
            nc.vector.memset(A4f[:, :, 9:72:9], 0.0)         # col 0, rows 1..7
            nc.vector.memset(A4f[:, :, 17:72:9], 0.0)        # col 8, rows 1..7

            s1acc = acts.tile([128, FPC * 2], F32)
            q1acc = acts.tile([128, FPC], F32)
            s2acc = acts.tile([128, 16], F32)
            q2acc = acts.tile([128, 16], F32)
            s3acc = acts.tile([128, 8], F32)
            q3acc = acts.tile([128, 8], F32)
            s4acc = acts.tile([128, 4], F32)
            q4acc = acts.tile([128, 4], F32)

            # ================ state embedding (own b): SE [64, 16] bf16
            # (independent of the CNN -- emit first so it fills startup gaps)
            pse0 = pbank()
            nc.tensor.matmul(pse0[0:64, 0, 0:T], swts[:], sts[:],
                             start=True, stop=True)
            SE = acts.tile([64, T], BF)
            nc.scalar.activation(SE[:], pse0[0:64, 0, 0:T], AF.Relu,
                                 bias=sbv[:, 0:1])

            # ================ conv1 + maxpool, frames in pairs
            for fp in range(FPC // 2):
                yf2 = ev.tile([128, 2, 2, 28, 28], BF, tag="yf2")
                for fi in range(2):
                    f = 2 * fp + fi
                    qf = qstream.tile([108, 28, 58], BF, tag="qf")
                    nc.sync.dma_start(qf[:], q_d[:, f, :, :])
                    pt = pbank()
                    for g in range(4):
                        ptv = pt[:, g, 0:392].rearrange("p (r x) -> p r x", r=7)
                        nc.tensor.matmul(ptv, w1p0[:], qf[:, 7 * g:7 * g + 7, 0:56],
                                         start=True, stop=False)
                        nc.tensor.matmul(ptv, w1p1[:], qf[:, 7 * g:7 * g + 7, 2:58],
                                         start=False, stop=True)
                    # copy out de-interleaved (even/odd x) + per-frame sums
                    for par in range(2):
                        nc.scalar.activation(
                            yf2[:, par, fi].rearrange("p (g r) u -> p g r u", g=4),
                            pt[:, :, par:392:2].rearrange("p g (r u) -> p g r u", r=7),
                            AF.Copy,
                            accum_out=s1acc[:, 2 * f + par:2 * f + par + 1])
                # x-direction pool for both frames at once (packed -> 4x STT)
                evm = yf2[:, 0].rearrange("p f y u -> p (f y) u")
                odm = yf2[:, 1].rearrange("p f y u -> p (f y) u")
                ax = ev.tile([128, 2, 28, 28], BF, tag="ax")
                axm = ax[:].rearrange("p f y u -> p (f y) u")
                nc.vector.scalar_tensor_tensor(
                    out=axm, in0=evm, scalar=1.0, in1=odm,
                    op0=ALU.mult, op1=ALU.max)
                nc.vector.scalar_tensor_tensor(
                    out=axm[:, :, 1:28], in0=axm[:, :, 1:28], scalar=1.0,
                    in1=odm[:, :, 0:27], op0=ALU.mult, op1=ALU.max)
                for fi in range(2):
                    f = 2 * fp + fi
                    # sum of squares (4x STT, dead scratch out)
                    sq1 = sqp.tile([128, 2, 784], BF, tag="sq1")
                    yfv = yf2[:, :, fi].rearrange("p par y u -> p par (y u)")
                    nc.vector.scalar_tensor_tensor(
                        out=sq1[:], in0=yfv, scalar=1.0, in1=yfv,
                        op0=ALU.mult, op1=ALU.mult,
                        accum_out=q1acc[:, f:f + 1])
                    # y-direction pool: rows y=2r+ry live as (partition ry, r)
                    pxb = ev.tile([64, 28, 28], BF, tag="pxb")
                    nc.vector.tensor_copy(pxb[:], ax[64:128, fi])
                    nc.vector.scalar_tensor_tensor(
                        out=A2[0:64, f, 1:29, 1:29], in0=ax[0:64, fi], scalar=1.0,
                        in1=pxb[:], op0=ALU.mult, op1=ALU.max)
                    nc.vector.scalar_tensor_tensor(
                        out=A2[0:64, f, 2:29, 1:29], in0=A2[0:64, f, 2:29, 1:29],
                        scalar=1.0, in1=pxb[:, 0:27, :], op0=ALU.mult, op1=ALU.max)

            # ---------------- BN helpers
            def bn_finalize(s_red, q_red, gb, cnt, P, K, lname):
                m = misc.tile([P, K], F32, tag=lname + "_m")
                nc.vector.tensor_scalar_mul(m[:], s_red[:], 1.0 / cnt)
                msq = misc.tile([P, K], F32, tag=lname + "_msq")
                nc.vector.tensor_mul(msq[:], m[:], m[:])
                v = misc.tile([P, K], F32, tag=lname + "_v")
                nc.vector.scalar_tensor_tensor(out=v[:], in0=q_red[:], scalar=1.0 / cnt,
                                               in1=msq[:], op0=ALU.mult,
                                               op1=ALU.subtract)
                ve = misc.tile([P, K], F32, tag=lname + "_ve")
                nc.vector.tensor_scalar_add(ve[:], v[:], float(BN_EPS))
                rc = misc.tile([P, K], F32, tag=lname + "_rc")
                nc.vector.reciprocal(rc[:], ve[:])
                rinv = misc.tile([P, K], F32, tag=lname + "_rinv")
                nc.scalar.activation(rinv[:], rc[:], AF.Sqrt)
                a = misc.tile([P, K], F32, tag=lname + "_a")
                nc.vector.tensor_mul(a[:], rinv[:], gb[:, 0:K])
                tmv = misc.tile([P, K], F32, tag=lname + "_tmv")
                nc.vector.tensor_mul(tmv[:], m[:], a[:])
                b = misc.tile([P, K], F32, tag=lname + "_b")
                nc.vector.tensor_sub(b[:], gb[:, K:2 * K], tmv[:])
                return a, b

            def allreduce(idx, src, P, K, lname):
                if not use_cc:
                    return src
                nc.sync.dma_start(cc_i[idx][:], src[:])
                nc.gpsimd.collective_compute(
                    "AllReduce", ALU.add, ins=[cc_i[idx][:]], outs=[cc_o[idx][:]],
                    replica_groups=[core_ids])
                g = misc.tile([P, K], F32, tag=lname + "_g")
                nc.sync.dma_start(g[:], cc_o[idx][:])
                return g

            # ---------------- BN1 -> A2
            s1r = misc.tile([128, 1], F32, tag="s1r")
            nc.vector.reduce_sum(s1r[:], s1acc[:], axis=AX.X)
            q1r = misc.tile([128, 1], F32, tag="q1r")
            nc.vector.reduce_sum(q1r[:], q1acc[:], axis=AX.X)
            st1 = misc.tile([64, 2], F32, tag="st1")
            s1hi = misc.tile([64, 2], F32, tag="s1hi")
            nc.vector.tensor_copy(s1hi[:, 0:1], s1r[64:128, :])
            nc.vector.tensor_copy(s1hi[:, 1:2], q1r[64:128, :])
            nc.vector.tensor_add(st1[:, 0:1], s1r[0:64, :], s1hi[:, 0:1])
            nc.vector.tensor_add(st1[:, 1:2], q1r[0:64, :], s1hi[:, 1:2])
            if use_cc:
                # fold 0*warm_g into the stats so the warmup collective is
                # not dead code (also serializes BN1 after the warmup).
                nc.vector.scalar_tensor_tensor(
                    out=st1[:, 0:1], in0=warm_g[:], scalar=0.0,
                    in1=st1[:, 0:1], op0=ALU.mult, op1=ALU.add)
            g1 = allreduce(0, st1, 64, 2, "l1")
            a1, b1 = bn_finalize(g1[:, 0:1], g1[:, 1:2], gb1, CNT[0], 64, 1, "l1")

            # ================ BN1-relu + x-shifted dup + conv2, interleaved
            # per 8-frame block so conv2 starts as soon as block 0 is ready.
            for blk in range(4):
                f0 = blk * 8
                nc.scalar.activation(A2[0:64, f0:f0 + 8, 1:29, 1:29],
                                     A2[0:64, f0:f0 + 8, 1:29, 1:29],
                                     AF.Relu, bias=b1[:, 0:1], scale=a1[:, 0:1])
                # dup: one flat packed copy (col 29 of src is zeroed; wrapped
                # cells land where conv2 never reads)
                nc.vector.tensor_copy(A2f[64:128, f0:f0 + 8, 0:870],
                                      A2f[0:64, f0:f0 + 8, 1:871])
                for g in range(blk * 4, blk * 4 + 4):
                    fg = g * 2
                    pt = pbank()
                    ptv = pt[:, 0, 0:392].rearrange("p (f y x) -> p f y x",
                                                    f=2, y=14)
                    for ky in range(3):
                        nc.tensor.matmul(ptv, w2A[:, ky, :],
                                         A2[:, fg:fg + 2, ky:ky + 28:2, 0:28:2],
                                         start=(ky == 0), stop=False)
                        nc.tensor.matmul(ptv, w2B[:, ky, :],
                                         A2[0:64, fg:fg + 2, ky:ky + 28:2, 2:30:2],
                                         start=False, stop=(ky == 2))
                    nc.scalar.activation(A3[:, fg:fg + 2, 1:15, 1:15], ptv,
                                         AF.Copy, accum_out=s2acc[:, g:g + 1])
                    sq2 = sqp.tile([128, 2, 14, 14], BF, tag="sq2")
                    nc.scalar.activation(sq2[:], A3[:, fg:fg + 2, 1:15, 1:15],
                                         AF.Square, accum_out=q2acc[:, g:g + 1])

            st2 = misc.tile([128, 2], F32, tag="st2")
            nc.vector.reduce_sum(st2[:, 0:1], s2acc[:], axis=AX.X)
            nc.vector.reduce_sum(st2[:, 1:2], q2acc[:], axis=AX.X)
            g2 = allreduce(1, st2, 128, 2, "l2")
            a2, b2 = bn_finalize(g2[:, 0:1], g2[:, 1:2], gb2, CNT[1], 128, 1, "l2")

            # ================ BN2-relu + conv3 interleaved per 8-frame block
            for g in range(4):
                f0 = g * 8
                nc.scalar.activation(A3[:, f0:f0 + 8, 1:15, 1:15],
                                     A3[:, f0:f0 + 8, 1:15, 1:15],
                                     AF.Relu, bias=b2[:, 0:1], scale=a2[:, 0:1])
                for m in range(2):
                    pt = pbank()
                    ptv = pt[:, 0, 0:392].rearrange("p (f y x) -> p f y x",
                                                    f=8, y=7)
                    for tap in range(9):
                        ky, kx = tap // 3, tap % 3
                        nc.tensor.matmul(ptv, w3s[:, tap, m, :],
                                         A3[:, f0:f0 + 8, ky:ky + 14:2, kx:kx + 14:2],
                                         start=(tap == 0), stop=(tap == 8))
                    ci = m * 4 + g
                    nc.scalar.activation(A4[:, m, f0:f0 + 8, 1:8, 1:8], ptv,
                                         AF.Copy, accum_out=s3acc[:, ci:ci + 1])
                    sq3 = sqp.tile([128, 8, 7, 7], BF, tag="sq3")
                    nc.scalar.activation(sq3[:], A4[:, m, f0:f0 + 8, 1:8, 1:8],
                                         AF.Square, accum_out=q3acc[:, ci:ci + 1])

            st3 = misc.tile([128, 4], F32, tag="st3")
            for m in range(2):
                nc.vector.reduce_sum(st3[:, m:m + 1], s3acc[:, m * 4:(m + 1) * 4],
                                     axis=AX.X)
                nc.vector.reduce_sum(st3[:, 2 + m:3 + m], q3acc[:, m * 4:(m + 1) * 4],
                                     axis=AX.X)
            g3 = allreduce(2, st3, 128, 4, "l3")
            a3, b3 = bn_finalize(g3[:, 0:2], g3[:, 2:4], gb3, CNT[2], 128, 2, "l3")
            for h in range(2):
                nc.scalar.activation(
                    A4[:, h, :, 1:8, 1:8], A4[:, h, :, 1:8, 1:8],
                    AF.Relu, bias=b3[:, h:h + 1], scale=a3[:, h:h + 1])

            # ================ conv4 (h-outer so m=0 can start after relu h=0)
            for m in range(4):
                pt = pbank()
                ptv = pt[:, 0, :].rearrange("p (f y x) -> p f y x", f=FPC, y=4)
                first = True
                for h in range(2):
                    for tap in range(9):
                        ky, kx = tap // 3, tap % 3
                        nc.tensor.matmul(ptv, w4s[:, tap, h, m, :],
                                         A4[:, h, :, ky:ky + 7:2, kx:kx + 7:2],
                                         start=first, stop=(tap == 8 and h == 1))
                        first = False
                nc.scalar.activation(
                    Y4[:, m, :, :].rearrange("p f (y x) -> p f y x", y=4),
                    ptv, AF.Copy, accum_out=s4acc[:, m:m + 1])
                sq4 = sqp.tile([128, FPC, 16], BF, tag="sq4")
                nc.vector.scalar_tensor_tensor(
                    out=sq4[:], in0=Y4[:, m, :, :], scalar=1.0, in1=Y4[:, m, :, :],
                    op0=ALU.mult, op1=ALU.mult,
                    accum_out=q4acc[:, m:m + 1])

            st4 = misc.tile([128, 8], F32, tag="st4")
            nc.vector.tensor_copy(st4[:, 0:4], s4acc[:])
            nc.vector.tensor_copy(st4[:, 4:8], q4acc[:])
            g4 = allreduce(3, st4, 128, 8, "l4")
            a4, b4 = bn_finalize(g4[:, 0:4], g4[:, 4:8], gb4, CNT[3], 128, 4, "l4")

            # BN4+relu -> avgpool(16px) -> image-mean -> feats [128,(4m,16t)]
            S4 = acts.tile([128, 4, FPC], F32)
            for m in range(4):
                r4m = ev.tile([128, FPC, 16], F32, tag="r4m")
                nc.scalar.activation(r4m[:], Y4[:, m, :, :], AF.Relu,
                                     bias=b4[:, m:m + 1], scale=a4[:, m:m + 1])
                nc.vector.reduce_sum(S4[:, m, :], r4m[:], axis=AX.X)
            fsum = acts.tile([128, 4, T], F32)
            nc.vector.tensor_add(fsum[:], S4[:, :, 0:T], S4[:, :, T:FPC])
            FIb = acts.tile([128, 4, T], BF)
            nc.vector.tensor_scalar_mul(FIb[:], fsum[:], 1.0 / 32.0)

            # ================ xin[p, t, m] = (Win @ step_inputs)_t + b_res
            xin = acts.tile([128, T, 8], F32)
            for m in range(8):
                pxt = pbank()
                px2 = pxt[:, 0, 0:T]
                for ki in range(4):
                    nc.tensor.matmul(px2, wis[:, ki, m, :], FIb[:, ki, :],
                                     start=(ki == 0), stop=False)
                nc.tensor.matmul(px2, wss[:, m, :], SE[:], start=False, stop=True)
                nc.scalar.activation(xin[:, :, m], px2, AF.Identity,
                                     bias=bres[:, m:m + 1])

            # ================ reservoir scan (own b), s = 2r form:
            # s_t = 0.5*s_{t-1} + tanh(xin_t + (0.5*Wres) s_{t-1})
            # xin is injected into PSUM via an fp32 identity matmul so the
            # scalar engine can tanh straight out of the bank.
            s_hist = acts.tile([128, T, 8], F32)
            sb16 = acts.tile([128, T, 8], BF)
            szero = acts.tile([128, 8], F32)
            szero16 = acts.tile([128, 8], BF)
            nc.vector.memset(szero[:], 0.0)
            nc.vector.memset(szero16[:], 0.0)
            for t in range(T):
                prt = pbank()
                nc.tensor.matmul(prt[:, 0, 0:8], idf[:], xin[:, t, :],
                                 start=True, stop=False, skip_group_check=True)
                for m in range(8):
                    for k in range(8):
                        rhs = (szero16[:, k:k + 1] if t == 0
                               else sb16[:, t - 1, k:k + 1])
                        nc.tensor.matmul(prt[:, 0, m:m + 1], wrs[:, k, m, :],
                                         rhs, start=False,
                                         stop=(m == 7 and k == 7),
                                         skip_group_check=True)
                th = ev.tile([128, 8], F32, tag="scan_th")
                nc.scalar.activation(th[:], prt[:, 0, 0:8], AF.Tanh)
                sprev = szero[:] if t == 0 else s_hist[:, t - 1, :]
                nc.vector.scalar_tensor_tensor(
                    out=s_hist[:, t, :], in0=sprev, scalar=0.5, in1=th[:],
                    op0=ALU.mult, op1=ALU.add)
                nc.vector.tensor_copy(sb16[:, t, :], s_hist[:, t, :])
                if t == T // 2 - 1:
                    # first-half AllGather overlaps the rest of the scan
                    nc.sync.dma_start(rg1_i[:], s_hist[:, 0:T // 2, :])
                    nc.gpsimd.collective_compute(
                        "AllGather", ALU.bypass, ins=[rg1_i[:]],
                        outs=[rg1_o[:]], replica_groups=[core_ids])

            # ================ head. hW1 is sharded by output rows; every core
            # computes its 64-row slice for ALL batches from the AllGathered
            # reservoir histories, then a masked AllReduce redistributes each
            # batch's full H.
            nc.sync.dma_start(rg2_i[:], s_hist[:, T // 2:T, :])
            nc.gpsimd.collective_compute(
                "AllGather", ALU.bypass, ins=[rg2_i[:]], outs=[rg2_o[:]],
                replica_groups=[core_ids])
            Rf = acts.tile([128, n_cores, T, 8], F32)
            Rb16 = acts.tile([128, n_cores, T, 8], BF)
            nc.sync.dma_start(Rf[:, :, 0:T // 2, :],
                              rg1_o[:].rearrange("b p t m -> p b t m"))
            nc.vector.tensor_copy(Rb16[:, :, 0:T // 2, :], Rf[:, :, 0:T // 2, :])
            pht = pbank()
            ph = pht[0:64, 0, 0:8]
            first = True
            for t in range(T // 2):
                for m in range(8):
                    nc.tensor.matmul(ph, h1s[:, t, m, :], Rb16[:, :, t, m],
                                     start=first, stop=False,
                                     skip_group_check=True)
                    first = False
            nc.sync.dma_start(Rf[:, :, T // 2:T, :],
                              rg2_o[:].rearrange("b p t m -> p b t m"))
            nc.vector.tensor_copy(Rb16[:, :, T // 2:T, :], Rf[:, :, T // 2:T, :])
            for t in range(T // 2, T):
                for m in range(8):
                    nc.tensor.matmul(ph, h1s[:, t, m, :], Rb16[:, :, t, m],
                                     start=False, stop=(t == T - 1 and m == 7),
                                     skip_group_check=True)
            hcs = misc.tile([64, 8], F32, tag="hcs")
            nc.scalar.activation(hcs[:], ph, AF.Identity, bias=hb1c[:, 0:1])
            # buf[p, r*8+b] = hcs[p, b] iff r == own rank, else 0
            hbuf = misc.tile([64, 64], F32, tag="hbuf")
            for r in range(n_cores):
                nc.vector.scalar_tensor_tensor(
                    out=hbuf[:, r * 8:(r + 1) * 8], in0=hcs[:],
                    scalar=maskr[:, r:r + 1],
                    in1=szero[0:64, 0:8], op0=ALU.mult, op1=ALU.add)
            gH = allreduce(4, hbuf, 64, 64, "lH")
            # select own batch's column: msel[p, r*8+b] = gH * (b == rank)
            msel = misc.tile([64, 64], F32, tag="msel")
            nc.vector.tensor_mul(msel[:], gH[:], maskb[:])
            Hsel = misc.tile([64, 8], F32, tag="Hsel")
            nc.vector.reduce_sum(
                Hsel[:], msel[:].rearrange("p (r b) -> p r b", b=8), axis=AX.X)
            H64 = misc.tile([64, 8], BF, tag="H64")
            nc.scalar.activation(H64[:], Hsel[:], AF.Relu)
            H128 = misc.tile([128, 4], BF, tag="H128")
            nc.vector.tensor_copy(H128[0:64, :], H64[:, 0:8:2])
            nc.vector.tensor_copy(H128[64:128, :], H64[:, 1:8:2])

            pot = pbank()
            po = pot[0:112, 0, 0:1]
            for j in range(4):
                nc.tensor.matmul(po, h2s[:, j, :], H128[:, j:j + 1],
                                 start=(j == 0), stop=(j == 3))
            OutS = acts.tile([112, 1], F32)
            nc.scalar.activation(OutS[:], po, AF.Identity, bias=hb2[:, 0:1])
            nc.sync.dma_start(out_d[:], OutS[:])
            if DEBUG_DUMPS:
                nc.sync.dma_start(dbg_xin[:], xin[:])
                nc.sync.dma_start(dbg_s[:], s_hist[:])
                nc.sync.dma_start(dbg_h[:], Hsel[:])
                nc.sync.dma_start(dbg_a2[:], A2[:, 0, :, :])
                nc.sync.dma_start(dbg_a3[:], A3[:, 0, :, :])
                nc.sync.dma_start(dbg_y4[:], Y4[:, :, 0, :].rearrange("p m x -> p m x"))

    nc.compile()
    return nc


# ----------------------------------------------------------------- host driver

_CACHE = {}


def make_in_map(inputs, core):
    b = core
    imgs = np.asarray(inputs["images_seq"], np.float32)[b].reshape(FPC, 3, 112, 112)
    wA, wB = pack_w2(inputs["w2"])
    wi, ws = pack_win(inputs["Win"])
    f32 = lambda x: np.asarray(x, np.float32)
    mask = np.zeros((64, 8), dtype=np.float32)
    mask[:, core] = 1.0
    maskb = np.zeros((64, 8, 8), dtype=np.float32)
    maskb[:, :, core] = 1.0
    w1p0, w1p1 = pack_w1_d4(inputs["w1"])
    d = {
        "qim": conv1_q(imgs),
        "w1p0": w1p0, "w1p1": w1p1,
        "idf32": np.eye(128, dtype=np.float32),
        "w2A": wA, "w2B": wB,
        "w3p": pack_w3(inputs["w3"]),
        "w4p": pack_w4(inputs["w4"]),
        "wip": wi, "wsp": ws,
        "wrp": pack_wres(inputs["Wres"]),
        "h1p": pack_hw1_slice(inputs["hW1"], core),
        "h2p": pack_hw2(inputs["hW2"]),
        "stT": _bf(f32(inputs["state_seq"])[b].T),
        "swT": _bf(f32(inputs["sW"]).T),
        "gb1": np.stack([f32(inputs["g1"]), f32(inputs["be1"])], axis=1),
        "gb2": np.stack([f32(inputs["g2"]), f32(inputs["be2"])], axis=1),
        "gb3": np.concatenate([f32(inputs["g3"]).reshape(2, 128).T,
                               f32(inputs["be3"]).reshape(2, 128).T], axis=1),
        "gb4": np.concatenate([f32(inputs["g4"]).reshape(4, 128).T,
                               f32(inputs["be4"]).reshape(4, 128).T], axis=1),
        "sbv": f32(inputs["sb"]).reshape(64, 1),
        "bres": f32(inputs["b_res"]).reshape(8, 128).T.copy(),
        "hb1c": f32(inputs["hb1"])[64 * core:64 * (core + 1)].reshape(64, 1),
        "hb2p": f32(inputs["hb2"]).reshape(112, 1),
        "maskr": mask,
        "maskb": maskb.reshape(64, 64),
    }
    return d


def run(inputs, n_cores=8, **kw):
    core_ids = list(range(n_cores))
    if n_cores not in _CACHE:
        _CACHE[n_cores] = build_program(n_cores)
    nc = _CACHE[n_cores]
    in_maps = [make_in_map(inputs, c) for c in core_ids]
    res = run_bass_kernel_spmd(nc, in_maps, core_ids, **kw)
    rows = [np.asarray(res.results[c]["out"], np.float32).reshape(112)
            for c in core_ids]
    return np.stack(rows, axis=0), res


def kernel(**inputs):
    out, _ = run(inputs, n_cores=8)
    return out.reshape(8, 8, 14)
